# revision 16
# baseline (speedup 1.0000x reference)
"""DGL capsule routing layer on 8 trn2 NeuronCores (Bass/Tile) — v3.

Math per iteration (b0 = 0):
    c = softmax(b, axis=out); s = einsum('io,iof->of', c, uh)
    v = squash(s); b += einsum('iof,of->io', uh, v)
Output: final v [OUT, F].

b_t = uh . w_{t-1} with w = cumulative v, so b is recomputed per pass.

v3 layout: uh cached in SBUF as bf16 with CHUNK-MAJOR f-outer columns:
flat col g = q*4096 + f*256 + o_l  (o = q*256 + o_l, q in 0..4).
Each pass-1 staging chunk (o-range q) converts into one CONTIGUOUS
4096-col span of the cache, so tile subtile-deps are exact and the pass-1
s matmuls (one [1,256] PSUM segment-group of 4 block-matmuls per (q,f))
start as soon as each chunk-set lands instead of after the whole load.
s / b / e / pt / AR all use this chunk-major flat order consistently:
  * AR payload ar[g] = s in chunk-major order, bf16, 32 KiB.
  * post-AR p-major [128,128] tiles: p = q*32 + f*2 + hi, free = lo
    (o = q*256 + hi*128 + lo); squash cross-partition f-sum and sc
    re-broadcast are one-hot PE matmuls (oh1: m=(p//32)*2+p%2,
    oh2: p'=(m//32)*2+m%2).
  * w accumulates bf16 in the same p-major tile; p-major flat IS the
    chunk-major flat, so the DRAM bounce + partition-stride-0 broadcast
    into w_fo line up with the uh cache for the next pass's tm mul.
Passes >= 2 per 128-i block: tm = uh*w as two f-half TT muls (strided
[p,4,2048] views, 2x_1p), b = halving tree over f with the t8 level on
DVE and t4/t2/b on GpSimd (frees ~15us DVE per pass), e = exp(b) on ACT
with fused denominator, rinv folded into the PE stationary (bf16).
s partials: per f-plane, pt = e*uh (TT, o-ordered out), two [1,512]
window matmuls PSUM-accumulating over the 4 i-blocks.
"""

import numpy as np
from contextlib import ExitStack

import concourse.bass as bass
import concourse.mybir as mybir
import concourse.tile as tile
from concourse import bacc
from concourse import bass_utils

F32 = mybir.dt.float32
BF16 = mybir.dt.bfloat16
AF = mybir.ActivationFunctionType
AO = mybir.AluOpType

IN_NODES, OUT_NODES, F_SIZE = 4096, 1024, 16
CORES = 8
I_LOC = IN_NODES // CORES          # 512 in-nodes per core
ROW = OUT_NODES * F_SIZE           # 16384 values per in-node row
P = 128
NBLK = I_LOC // P                  # 4 i-blocks per core
QT = 4096                          # staging chunk = cache chunk (cols)
NQT = ROW // QT                    # 4 chunks
O = OUT_NODES
H = ROW // 2                       # 8192
SEG = 256                          # o-cols per (q,f) cache segment


def _body(nc, tc, uh, v_out, R, rg):
    uh_t = uh.rearrange("(n p) r -> n p r", p=P)   # [NBLK, 128, 16384] f32

    with ExitStack() as ctx:
        persist = ctx.enter_context(tc.tile_pool(name="persist", bufs=1))
        scp = ctx.enter_context(tc.tile_pool(name="scp", bufs=2))
        smp = ctx.enter_context(tc.tile_pool(name="smp", bufs=1))
        psp = ctx.enter_context(tc.tile_pool(name="psp", bufs=3, space="PSUM"))
        dram = ctx.enter_context(tc.tile_pool(name="dram", bufs=2, space="DRAM"))

        # --- persistent tiles -------------------------------------------
        uhb = [persist.tile([P, ROW], BF16, name=f"uhb{k}", tag=f"uhb{k}")
               for k in range(NBLK)]
        w_fo = None
        if R > 1:
            w_fo = persist.tile([P, ROW], BF16, name="w_fo")
        c0 = persist.tile([P, 1], BF16, name="c0")
        nc.vector.memset(c0, 1.0 / OUT_NODES)
        # one-hot stationaries for squash in the chunk-major p-mapping:
        # p = q*32 + f*2 + hi  ->  m = q*2 + hi = (p//32)*2 + p%2
        pidx = np.arange(P)
        m_of_p = (pidx // 32) * 2 + (pidx % 2)
        oh1_d = nc.inline_tensor(
            (m_of_p[:, None] == np.arange(8)[None, :]).astype('bfloat16'),
            name="oh1d")
        oh2_d = nc.inline_tensor(
            (np.arange(8)[:, None] == m_of_p[None, :]).astype('bfloat16'),
            name="oh2d")
        oh1 = persist.tile([P, 8], BF16, name="oh1")
        nc.sync.dma_start(oh1, oh1_d.ap())
        oh2 = persist.tile([8, P], BF16, name="oh2")
        nc.sync.dma_start(oh2, oh2_d.ap())

        w_acc_prev = None

        def fhalf(tile_, h):
            # strided f-half view: [p, 4 chunks, 2048] (f<8 or f>=8)
            return tile_.rearrange("p (q c) -> p q c", c=QT)[
                :, :, h * (QT // 2):(h + 1) * (QT // 2)]

        for t in range(1, R + 1):
            ar_in = dram.tile([ROW], BF16, tag="ar_in")
            ar_v = ar_in.rearrange("(q f c) -> q f c", q=NQT, f=F_SIZE)
            if t == 1:
                # ---- pass 1: stream, convert, and matmul per chunk-set ----
                for q in range(NQT):
                    for blk in range(NBLK):
                        st = scp.tile([P, QT], F32, tag="sc", name="st")
                        nc.sync.dma_start(
                            st, uh_t[blk, :, q * QT:(q + 1) * QT])
                        dst = uhb[blk][:, q * QT:(q + 1) * QT].rearrange(
                            "p (f c) -> p f c", f=F_SIZE)
                        nc.vector.tensor_copy(
                            dst, st.rearrange("p (o f) -> p f o", f=F_SIZE))
                    # per f-quad: one [1,1024] psum, 4 segment-groups
                    for fq in range(F_SIZE // 4):
                        ps = psp.tile([1, O], F32, tag="ps1", name="ps")
                        for j in range(4):
                            f = fq * 4 + j
                            for blk in range(NBLK):
                                nc.tensor.matmul(
                                    ps[:, j * SEG:(j + 1) * SEG], c0,
                                    uhb[blk][:, q * QT + f * SEG:
                                             q * QT + (f + 1) * SEG],
                                    start=(blk == 0), stop=(blk == NBLK - 1),
                                    skip_group_check=True)
                        fl = smp.tile([1, O], BF16, tag="bfl", bufs=2,
                                      name="fl")
                        nc.scalar.copy(fl, ps)
                        # quad spans contiguous chunk-major cols
                        nc.sync.dma_start(
                            ar_in[q * QT + fq * O:q * QT + (fq + 1) * O], fl)
                rbs = [c0] * NBLK
                e2s = None
            else:
                # ---- passes >= 2: b, e, rinv per block from SBUF cache ----
                rbs, e2s = [], []
                for blk in range(NBLK):
                    tmA = scp.tile([P, H], BF16, tag="sc", name="tmA")
                    tmB = scp.tile([P, H], BF16, tag="sc", name="tmB")
                    tmAv = tmA.rearrange("p (q c) -> p q c", c=QT // 2)
                    tmBv = tmB.rearrange("p (q c) -> p q c", c=QT // 2)
                    uv = uhb[blk].rearrange("p (q c) -> p q c", c=QT)
                    wv = w_fo.rearrange("p (q c) -> p q c", c=QT)
                    for u in range(2):
                        sl = slice(u * 1024, (u + 1) * 1024)
                        nc.vector.tensor_mul(
                            tmAv[:, :, sl], uv[:, :, sl], wv[:, :, sl])
                    for u in range(2):
                        sl = slice(u * 1024, (u + 1) * 1024)
                        s2 = slice(2048 + u * 1024, 2048 + (u + 1) * 1024)
                        nc.vector.tensor_mul(
                            tmBv[:, :, sl], uv[:, :, s2], wv[:, :, s2])
                    # halving tree over f (in-place, all on DVE)
                    nc.vector.tensor_add(tmA, tmA, tmB)
                    nc.vector.tensor_add(
                        tmAv[:, :, 0:1024], tmAv[:, :, 0:1024],
                        tmAv[:, :, 1024:2048])
                    nc.vector.tensor_add(
                        tmAv[:, :, 0:512], tmAv[:, :, 0:512],
                        tmAv[:, :, 512:1024])
                    b = smp.tile([P, O], BF16, tag="bfl", bufs=2,
                                 name="b")
                    nc.vector.tensor_add(
                        b.rearrange("p (q c) -> p q c", c=SEG),
                        tmAv[:, :, 0:SEG], tmAv[:, :, SEG:2 * SEG])
                    e2 = smp.tile([P, O], BF16, tag=f"e2_{blk}", name="e2")
                    den = smp.tile([P, 1], F32, tag="den", name="den")
                    nc.scalar.activation(e2, b, AF.Exp, accum_out=den)
                    rinv = smp.tile([P, 1], F32, tag="rinv", name="rinv")
                    nc.vector.reciprocal(rinv, den)
                    rb = smp.tile([P, 1], BF16, tag=f"rb_{blk}", name="rb")
                    nc.vector.tensor_copy(rb, rinv)
                    rbs.append(rb)
                    e2s.append(e2)

                # ---- s partials: per f-plane, PSUM-accumulate over blocks
                for f in range(F_SIZE):
                    ps = psp.tile([1, O], F32, tag="ps1", name="ps",
                                  padded_shape=[P, O])
                    for blk in range(NBLK):
                        pt = scp.tile([P, O], BF16, tag="sc", name="pt")
                        nc.vector.tensor_mul(
                            pt.rearrange("p (q c) -> p q c", c=SEG),
                            uhb[blk].rearrange("p (q c) -> p q c", c=QT)[
                                :, :, f * SEG:(f + 1) * SEG],
                            e2s[blk].rearrange("p (q c) -> p q c", c=SEG))
                        for w0 in (0, 512):
                            nc.tensor.matmul(
                                ps[:, w0:w0 + 512], rbs[blk],
                                pt[:, w0:w0 + 512],
                                start=(blk == 0), stop=(blk == NBLK - 1),
                                skip_group_check=True)
                    fl = smp.tile([1, O], BF16, tag="bfl", bufs=2,
                                  name="fl")
                    nc.scalar.copy(fl, ps)
                    # s[f, o] o-ordered -> chunk-major ar positions
                    # (dest strided [q,c] view flattens to o-order)
                    nc.sync.dma_start(ar_v[:, f], fl)

            ar_out = dram.tile([ROW], BF16, tag="ar_out")
            nc.gpsimd.collective_compute(
                "AllReduce", AO.add, replica_groups=rg,
                ins=[ar_in.opt()], outs=[ar_out.opt()],
            )

            # ---- squash in p-major layout: p = q*32+f*2+hi, free = lo ----
            sld = smp.tile([P, P], BF16, tag="sld", name="sld")
            nc.sync.dma_start(sld, ar_out.rearrange("(p q) -> p q", p=P))
            ssq = smp.tile([P, P], BF16, tag="ssq", name="ssq")
            nc.vector.tensor_mul(ssq, sld, sld)
            sqps = psp.tile([8, P], F32, tag="sqps", bufs=1, name="sqps")
            nc.tensor.matmul(sqps, oh1, ssq, start=True, stop=True,
                             skip_group_check=True)
            sq = smp.tile([8, P], BF16, tag="sq", name="sq")
            nc.scalar.copy(sq, sqps)
            # sqrt(sq) via exp(0.5*ln) + one Newton step (exp/ln table set)
            lnq = smp.tile([8, P], BF16, tag="lnq", name="lnq")
            nc.scalar.activation(lnq, sq, AF.Ln)
            y = smp.tile([8, P], BF16, tag="y", name="y")
            nc.scalar.activation(y, lnq, AF.Exp, scale=0.5)
            ry = smp.tile([8, P], BF16, tag="ry", name="ry")
            with nc.allow_low_precision(reason="bf16 squash chain"):
                nc.vector.reciprocal(ry, y)
            t1 = smp.tile([8, P], BF16, tag="t1", name="t1")
            nc.vector.tensor_mul(t1, sq, ry)
            nc.vector.tensor_add(t1, t1, y)        # t1 = sq/y + y = 2*sqrt
            d2 = smp.tile([8, P], BF16, tag="lnq", name="d2")
            nc.vector.tensor_scalar(d2, sq, 1.0, 2.0, AO.add, AO.mult)
            rd = smp.tile([8, P], BF16, tag="ry", name="rd")
            with nc.allow_low_precision(reason="bf16 squash chain"):
                nc.vector.reciprocal(rd, d2)       # rd = 0.5/(1+sq)
            sc = smp.tile([8, P], BF16, tag="sq", name="sc")
            nc.vector.tensor_mul(sc, t1, rd)       # sqrt(sq)/(1+sq)
            srps = psp.tile([P, P], F32, tag="srps", bufs=1, name="srps")
            nc.tensor.matmul(srps, oh2, sc, start=True, stop=True,
                             skip_group_check=True)
            v_sb = smp.tile([P, P], BF16, tag="v_sb", name="v_sb")
            nc.vector.tensor_mul(v_sb, sld, srps)

            if t == R:
                nc.sync.dma_start(v_out, v_sb)
            else:
                w_acc = smp.tile([P, P], BF16, tag="w_acc", bufs=2,
                                 name="w_acc")
                if t == 1:
                    nc.scalar.copy(w_acc, v_sb)
                else:
                    nc.vector.tensor_add(w_acc, w_acc_prev, v_sb)
                w_acc_prev = w_acc
                w_dram = dram.tile([ROW], BF16, tag="w_dram")
                nc.sync.dma_start(
                    w_dram.rearrange("(p q) -> p q", p=P), w_acc)
                wd_b = w_dram.unsqueeze(0)
                wd_v = wd_b.rearrange("x (q c) -> x q c", c=QT)
                w_fo_v = w_fo.rearrange("p (q c) -> p q c", c=QT)
                for u in range(4):
                    sl = slice(u * (QT // 4), (u + 1) * (QT // 4))
                    nc.sync.dma_start(
                        w_fo_v[:, :, sl],
                        wd_v[:, :, sl].broadcast_to([P, NQT, QT // 4]))


def _build(routing_num: int):
    R = int(routing_num)
    assert R >= 1
    nc = bacc.Bacc(
        "TRN2", target_bir_lowering=False, debug=False, num_devices=CORES)
    uh = nc.dram_tensor("uh", [I_LOC, ROW], F32, kind="ExternalInput")
    v_out = nc.dram_tensor("v_out", [P, P], BF16, kind="ExternalOutput")
    rg = [list(range(CORES))]
    with tile.TileContext(nc) as tc:
        _body(nc, tc, uh.ap(), v_out.ap(), R, rg)
    nc.compile()
    return nc


_CACHE: dict = {}


def _get_nc(routing_num: int):
    R = int(routing_num)
    if R not in _CACHE:
        _CACHE[R] = _build(R)
    return _CACHE[R]


def _shard(u_hat: np.ndarray):
    uh = np.ascontiguousarray(np.asarray(u_hat, dtype=np.float32))
    assert uh.shape == (IN_NODES * OUT_NODES, F_SIZE), uh.shape
    uh = uh.reshape(IN_NODES, ROW)
    return [
        {"uh": np.ascontiguousarray(uh[k * I_LOC:(k + 1) * I_LOC])}
        for k in range(CORES)
    ]


def run(u_hat, routing_num, trace=False):
    nc = _get_nc(routing_num)
    in_maps = _shard(u_hat)
    res = bass_utils.run_bass_kernel_spmd(
        nc, in_maps, core_ids=list(range(CORES)), trace=trace)
    return res


def _unpack(v_pm) -> np.ndarray:
    # [128,128] p-major bf16, p = q*32 + f*2 + hi, free = lo
    # o = q*256 + hi*128 + lo  ->  [1024, 16] f32
    v = np.asarray(v_pm).astype(np.float32).reshape(NQT, F_SIZE, 2, P)
    return np.ascontiguousarray(
        v.transpose(0, 2, 3, 1).reshape(OUT_NODES, F_SIZE))


def kernel(u_hat, routing_num):
    res = run(u_hat, routing_num, trace=False)
    return _unpack(res.results[0]["v_out"])


# revision 17
# speedup vs baseline: 1.1607x; 1.1607x over previous
"""DGL capsule routing layer on 8 trn2 NeuronCores (Bass/Tile) — v3.

Math per iteration (b0 = 0):
    c = softmax(b, axis=out); s = einsum('io,iof->of', c, uh)
    v = squash(s); b += einsum('iof,of->io', uh, v)
Output: final v [OUT, F].

b_t = uh . w_{t-1} with w = cumulative v, so b is recomputed per pass.

v3 layout: uh cached in SBUF as bf16 with CHUNK-MAJOR f-outer columns:
flat col g = q*4096 + f*256 + o_l  (o = q*256 + o_l, q in 0..4).
Each pass-1 staging chunk (o-range q) converts into one CONTIGUOUS
4096-col span of the cache, so tile subtile-deps are exact and the pass-1
s matmuls (one [1,256] PSUM segment-group of 4 block-matmuls per (q,f))
start as soon as each chunk-set lands instead of after the whole load.
s / b / e / pt / AR all use this chunk-major flat order consistently:
  * AR payload ar[g] = s in chunk-major order, bf16, 32 KiB.
  * post-AR p-major [128,128] tiles: p = q*32 + f*2 + hi, free = lo
    (o = q*256 + hi*128 + lo); squash cross-partition f-sum and sc
    re-broadcast are one-hot PE matmuls (oh1: m=(p//32)*2+p%2,
    oh2: p'=(m//32)*2+m%2).
  * w accumulates bf16 in the same p-major tile; p-major flat IS the
    chunk-major flat, so the DRAM bounce + partition-stride-0 broadcast
    into w_fo line up with the uh cache for the next pass's tm mul.
Passes >= 2 per 128-i block: tm = uh*w as two f-half TT muls (strided
[p,4,2048] views, 2x_1p), b = halving tree over f with the t8 level on
DVE and t4/t2/b on GpSimd (frees ~15us DVE per pass), e = exp(b) on ACT
with fused denominator, rinv folded into the PE stationary (bf16).
s partials: per f-plane, pt = e*uh (TT, o-ordered out), two [1,512]
window matmuls PSUM-accumulating over the 4 i-blocks.
"""

import numpy as np
from contextlib import ExitStack

import concourse.bass as bass
import concourse.mybir as mybir
import concourse.tile as tile
from concourse import bacc
from concourse import bass_utils

F32 = mybir.dt.float32
BF16 = mybir.dt.bfloat16
AF = mybir.ActivationFunctionType
AO = mybir.AluOpType

IN_NODES, OUT_NODES, F_SIZE = 4096, 1024, 16
CORES = 8
I_LOC = IN_NODES // CORES          # 512 in-nodes per core
ROW = OUT_NODES * F_SIZE           # 16384 values per in-node row
P = 128
NBLK = I_LOC // P                  # 4 i-blocks per core
QT = 4096                          # staging chunk = cache chunk (cols)
NQT = ROW // QT                    # 4 chunks
O = OUT_NODES
H = ROW // 2                       # 8192
SEG = 256                          # o-cols per (q,f) cache segment


def _body(nc, tc, uh, v_out, R, rg):
    uh_t = uh.rearrange("(n p) r -> n p r", p=P)   # [NBLK, 128, 16384] f32

    with ExitStack() as ctx:
        persist = ctx.enter_context(tc.tile_pool(name="persist", bufs=1))
        scp = ctx.enter_context(tc.tile_pool(name="scp", bufs=2))
        smp = ctx.enter_context(tc.tile_pool(name="smp", bufs=1))
        psp = ctx.enter_context(tc.tile_pool(name="psp", bufs=3, space="PSUM"))
        dram = ctx.enter_context(tc.tile_pool(name="dram", bufs=2, space="DRAM"))

        # --- persistent tiles -------------------------------------------
        uhb = [persist.tile([P, ROW], BF16, name=f"uhb{k}", tag=f"uhb{k}")
               for k in range(NBLK)]
        w_fo = None
        if R > 1:
            w_fo = persist.tile([P, ROW], BF16, name="w_fo")
        c0 = persist.tile([P, 1], BF16, name="c0")
        nc.vector.memset(c0, 1.0 / OUT_NODES)
        # one-hot stationaries for squash in the chunk-major p-mapping:
        # p = q*32 + f*2 + hi  ->  m = q*2 + hi = (p//32)*2 + p%2
        pidx = np.arange(P)
        m_of_p = (pidx // 32) * 2 + (pidx % 2)
        oh1_d = nc.inline_tensor(
            (m_of_p[:, None] == np.arange(8)[None, :]).astype('bfloat16'),
            name="oh1d")
        oh2_d = nc.inline_tensor(
            (np.arange(8)[:, None] == m_of_p[None, :]).astype('bfloat16'),
            name="oh2d")
        oh1 = persist.tile([P, 8], BF16, name="oh1")
        nc.sync.dma_start(oh1, oh1_d.ap())
        oh2 = persist.tile([8, P], BF16, name="oh2")
        nc.sync.dma_start(oh2, oh2_d.ap())

        w_acc_prev = None

        def fhalf(tile_, h):
            # strided f-half view: [p, 4 chunks, 2048] (f<8 or f>=8)
            return tile_.rearrange("p (q c) -> p q c", c=QT)[
                :, :, h * (QT // 2):(h + 1) * (QT // 2)]

        for t in range(1, R + 1):
            ar_in = dram.tile([ROW], BF16, tag="ar_in")
            ar_v = ar_in.rearrange("(q f c) -> q f c", q=NQT, f=F_SIZE)
            if t == 1:
                # ---- pass 1: stream, convert, and matmul per chunk-set ----
                for q in range(NQT):
                    for blk in range(NBLK):
                        st = scp.tile([P, QT], F32, tag="sc", name="st")
                        nc.sync.dma_start(
                            st, uh_t[blk, :, q * QT:(q + 1) * QT])
                        dst = uhb[blk][:, q * QT:(q + 1) * QT].rearrange(
                            "p (f c) -> p f c", f=F_SIZE)
                        nc.vector.tensor_copy(
                            dst, st.rearrange("p (o f) -> p f o", f=F_SIZE))
                    # per f-quad: one [1,1024] psum, 4 segment-groups
                    for fq in range(F_SIZE // 4):
                        ps = psp.tile([1, O], F32, tag="ps1", name="ps")
                        for j in range(4):
                            f = fq * 4 + j
                            for blk in range(NBLK):
                                nc.tensor.matmul(
                                    ps[:, j * SEG:(j + 1) * SEG], c0,
                                    uhb[blk][:, q * QT + f * SEG:
                                             q * QT + (f + 1) * SEG],
                                    start=(blk == 0), stop=(blk == NBLK - 1),
                                    skip_group_check=True)
                        fl = smp.tile([1, O], BF16, tag="bfl", bufs=2,
                                      name="fl")
                        nc.scalar.copy(fl, ps)
                        # quad spans contiguous chunk-major cols
                        nc.sync.dma_start(
                            ar_in[q * QT + fq * O:q * QT + (fq + 1) * O], fl)
                rbs = [c0] * NBLK
                e2s = None
            else:
                # ---- passes >= 2: b, e, rinv per block from SBUF cache ----
                rbs, e2s = [], []
                for blk in range(NBLK):
                    tmA = scp.tile([P, H], BF16, tag="sc", name="tmA")
                    tmB = scp.tile([P, H], BF16, tag="sc", name="tmB")
                    tmAv = tmA.rearrange("p (q c) -> p q c", c=QT // 2)
                    tmBv = tmB.rearrange("p (q c) -> p q c", c=QT // 2)
                    nc.vector.tensor_mul(tmAv, fhalf(uhb[blk], 0),
                                         fhalf(w_fo, 0))
                    nc.vector.tensor_mul(tmBv, fhalf(uhb[blk], 1),
                                         fhalf(w_fo, 1))
                    # halving tree over f (in-place, all on DVE)
                    nc.vector.tensor_add(tmA, tmA, tmB)
                    nc.vector.tensor_add(
                        tmAv[:, :, 0:1024], tmAv[:, :, 0:1024],
                        tmAv[:, :, 1024:2048])
                    nc.vector.tensor_add(
                        tmAv[:, :, 0:512], tmAv[:, :, 0:512],
                        tmAv[:, :, 512:1024])
                    b = smp.tile([P, O], BF16, tag="bfl", bufs=2,
                                 name="b")
                    nc.vector.tensor_add(
                        b.rearrange("p (q c) -> p q c", c=SEG),
                        tmAv[:, :, 0:SEG], tmAv[:, :, SEG:2 * SEG])
                    e2 = smp.tile([P, O], BF16, tag=f"e2_{blk}", name="e2")
                    den = smp.tile([P, 1], F32, tag="den", name="den")
                    nc.scalar.activation(e2, b, AF.Exp, accum_out=den)
                    rinv = smp.tile([P, 1], F32, tag="rinv", name="rinv")
                    nc.vector.reciprocal(rinv, den)
                    rb = smp.tile([P, 1], BF16, tag=f"rb_{blk}", name="rb")
                    nc.vector.tensor_copy(rb, rinv)
                    rbs.append(rb)
                    e2s.append(e2)

                # ---- s partials: per f-plane, PSUM-accumulate over blocks
                for f in range(F_SIZE):
                    ps = psp.tile([1, O], F32, tag="ps1", name="ps",
                                  padded_shape=[P, O])
                    for blk in range(NBLK):
                        pt = scp.tile([P, O], BF16, tag="sc", name="pt")
                        nc.vector.tensor_mul(
                            pt.rearrange("p (q c) -> p q c", c=SEG),
                            uhb[blk].rearrange("p (q c) -> p q c", c=QT)[
                                :, :, f * SEG:(f + 1) * SEG],
                            e2s[blk].rearrange("p (q c) -> p q c", c=SEG))
                        for w0 in (0, 512):
                            nc.tensor.matmul(
                                ps[:, w0:w0 + 512], rbs[blk],
                                pt[:, w0:w0 + 512],
                                start=(blk == 0), stop=(blk == NBLK - 1),
                                skip_group_check=True)
                    fl = smp.tile([1, O], BF16, tag="bfl", bufs=2,
                                  name="fl")
                    nc.scalar.copy(fl, ps)
                    # s[f, o] o-ordered -> chunk-major ar positions
                    # (dest strided [q,c] view flattens to o-order)
                    nc.sync.dma_start(ar_v[:, f], fl)

            ar_out = dram.tile([ROW], BF16, tag="ar_out")
            nc.gpsimd.collective_compute(
                "AllReduce", AO.add, replica_groups=rg,
                ins=[ar_in.opt()], outs=[ar_out.opt()],
            )

            # ---- squash in p-major layout: p = q*32+f*2+hi, free = lo ----
            sld = smp.tile([P, P], BF16, tag="sld", name="sld")
            nc.sync.dma_start(sld, ar_out.rearrange("(p q) -> p q", p=P))
            ssq = smp.tile([P, P], BF16, tag="ssq", name="ssq")
            nc.vector.tensor_mul(ssq, sld, sld)
            sqps = psp.tile([8, P], F32, tag="sqps", bufs=1, name="sqps")
            nc.tensor.matmul(sqps, oh1, ssq, start=True, stop=True,
                             skip_group_check=True)
            sq = smp.tile([8, P], BF16, tag="sq", name="sq")
            nc.scalar.copy(sq, sqps)
            # sqrt(sq) via exp(0.5*ln) + one Newton step (exp/ln table set)
            lnq = smp.tile([8, P], BF16, tag="lnq", name="lnq")
            nc.scalar.activation(lnq, sq, AF.Ln)
            y = smp.tile([8, P], BF16, tag="y", name="y")
            nc.scalar.activation(y, lnq, AF.Exp, scale=0.5)
            ry = smp.tile([8, P], BF16, tag="ry", name="ry")
            with nc.allow_low_precision(reason="bf16 squash chain"):
                nc.vector.reciprocal(ry, y)
            t1 = smp.tile([8, P], BF16, tag="t1", name="t1")
            nc.vector.tensor_mul(t1, sq, ry)
            nc.vector.tensor_add(t1, t1, y)        # t1 = sq/y + y = 2*sqrt
            d2 = smp.tile([8, P], BF16, tag="lnq", name="d2")
            nc.vector.tensor_scalar(d2, sq, 1.0, 2.0, AO.add, AO.mult)
            rd = smp.tile([8, P], BF16, tag="ry", name="rd")
            with nc.allow_low_precision(reason="bf16 squash chain"):
                nc.vector.reciprocal(rd, d2)       # rd = 0.5/(1+sq)
            sc = smp.tile([8, P], BF16, tag="sq", name="sc")
            nc.vector.tensor_mul(sc, t1, rd)       # sqrt(sq)/(1+sq)
            srps = psp.tile([P, P], F32, tag="srps", bufs=1, name="srps")
            nc.tensor.matmul(srps, oh2, sc, start=True, stop=True,
                             skip_group_check=True)
            v_sb = smp.tile([P, P], BF16, tag="v_sb", name="v_sb")
            nc.vector.tensor_mul(v_sb, sld, srps)

            if t == R:
                nc.sync.dma_start(v_out, v_sb)
            else:
                w_acc = smp.tile([P, P], BF16, tag="w_acc", bufs=2,
                                 name="w_acc")
                if t == 1:
                    nc.scalar.copy(w_acc, v_sb)
                else:
                    nc.vector.tensor_add(w_acc, w_acc_prev, v_sb)
                w_acc_prev = w_acc
                w_dram = dram.tile([ROW], BF16, tag="w_dram")
                nc.sync.dma_start(
                    w_dram.rearrange("(p q) -> p q", p=P), w_acc)
                wd_b = w_dram.unsqueeze(0)
                wd_v = wd_b.rearrange("x (q c) -> x q c", c=QT)
                for h in (0, 1):
                    sl = slice(h * (QT // 2), (h + 1) * (QT // 2))
                    nc.sync.dma_start(
                        fhalf(w_fo, h),
                        wd_v[:, :, sl].broadcast_to([P, NQT, QT // 2]))


def _build(routing_num: int):
    R = int(routing_num)
    assert R >= 1
    nc = bacc.Bacc(
        "TRN2", target_bir_lowering=False, debug=False, num_devices=CORES)
    uh = nc.dram_tensor("uh", [I_LOC, ROW], F32, kind="ExternalInput")
    v_out = nc.dram_tensor("v_out", [P, P], BF16, kind="ExternalOutput")
    rg = [list(range(CORES))]
    with tile.TileContext(nc) as tc:
        _body(nc, tc, uh.ap(), v_out.ap(), R, rg)
    nc.compile()
    return nc


_CACHE: dict = {}


def _get_nc(routing_num: int):
    R = int(routing_num)
    if R not in _CACHE:
        _CACHE[R] = _build(R)
    return _CACHE[R]


def _shard(u_hat: np.ndarray):
    uh = np.ascontiguousarray(np.asarray(u_hat, dtype=np.float32))
    assert uh.shape == (IN_NODES * OUT_NODES, F_SIZE), uh.shape
    uh = uh.reshape(IN_NODES, ROW)
    return [
        {"uh": np.ascontiguousarray(uh[k * I_LOC:(k + 1) * I_LOC])}
        for k in range(CORES)
    ]


def run(u_hat, routing_num, trace=False):
    nc = _get_nc(routing_num)
    in_maps = _shard(u_hat)
    res = bass_utils.run_bass_kernel_spmd(
        nc, in_maps, core_ids=list(range(CORES)), trace=trace)
    return res


def _unpack(v_pm) -> np.ndarray:
    # [128,128] p-major bf16, p = q*32 + f*2 + hi, free = lo
    # o = q*256 + hi*128 + lo  ->  [1024, 16] f32
    v = np.asarray(v_pm).astype(np.float32).reshape(NQT, F_SIZE, 2, P)
    return np.ascontiguousarray(
        v.transpose(0, 2, 3, 1).reshape(OUT_NODES, F_SIZE))


def kernel(u_hat, routing_num):
    res = run(u_hat, routing_num, trace=False)
    return _unpack(res.results[0]["v_out"])


# revision 18
# speedup vs baseline: 1.1863x; 1.0221x over previous
"""DGL capsule routing layer on 8 trn2 NeuronCores (Bass/Tile) — v3.

Math per iteration (b0 = 0):
    c = softmax(b, axis=out); s = einsum('io,iof->of', c, uh)
    v = squash(s); b += einsum('iof,of->io', uh, v)
Output: final v [OUT, F].

b_t = uh . w_{t-1} with w = cumulative v, so b is recomputed per pass.

v3 layout: uh cached in SBUF as bf16 with CHUNK-MAJOR f-outer columns:
flat col g = q*4096 + f*256 + o_l  (o = q*256 + o_l, q in 0..4).
Each pass-1 staging chunk (o-range q) converts into one CONTIGUOUS
4096-col span of the cache, so tile subtile-deps are exact and the pass-1
s matmuls (one [1,256] PSUM segment-group of 4 block-matmuls per (q,f))
start as soon as each chunk-set lands instead of after the whole load.
s / b / e / pt / AR all use this chunk-major flat order consistently:
  * AR payload ar[g] = s in chunk-major order, bf16, 32 KiB.
  * post-AR p-major [128,128] tiles: p = q*32 + f*2 + hi, free = lo
    (o = q*256 + hi*128 + lo); squash cross-partition f-sum and sc
    re-broadcast are one-hot PE matmuls (oh1: m=(p//32)*2+p%2,
    oh2: p'=(m//32)*2+m%2).
  * w accumulates bf16 in the same p-major tile; p-major flat IS the
    chunk-major flat, so the DRAM bounce + partition-stride-0 broadcast
    into w_fo line up with the uh cache for the next pass's tm mul.
Passes >= 2 per 128-i block: tm = uh*w as two f-half TT muls (strided
[p,4,2048] views, 2x_1p), b = halving tree over f with the t8 level on
DVE and t4/t2/b on GpSimd (frees ~15us DVE per pass), e = exp(b) on ACT
with fused denominator, rinv folded into the PE stationary (bf16).
s partials: per f-plane, pt = e*uh (TT, o-ordered out), two [1,512]
window matmuls PSUM-accumulating over the 4 i-blocks.
"""

import numpy as np
from contextlib import ExitStack

import concourse.bass as bass
import concourse.mybir as mybir
import concourse.tile as tile
from concourse import bacc
from concourse import bass_utils

F32 = mybir.dt.float32
BF16 = mybir.dt.bfloat16
AF = mybir.ActivationFunctionType
AO = mybir.AluOpType

IN_NODES, OUT_NODES, F_SIZE = 4096, 1024, 16
CORES = 8
I_LOC = IN_NODES // CORES          # 512 in-nodes per core
ROW = OUT_NODES * F_SIZE           # 16384 values per in-node row
P = 128
NBLK = I_LOC // P                  # 4 i-blocks per core
QT = 4096                          # staging chunk = cache chunk (cols)
NQT = ROW // QT                    # 4 chunks
O = OUT_NODES
H = ROW // 2                       # 8192
SEG = 256                          # o-cols per (q,f) cache segment


def _body(nc, tc, uh, v_out, R, rg):
    uh_t = uh.rearrange("(n p) r -> n p r", p=P)   # [NBLK, 128, 16384] f32

    with ExitStack() as ctx:
        persist = ctx.enter_context(tc.tile_pool(name="persist", bufs=1))
        scp = ctx.enter_context(tc.tile_pool(name="scp", bufs=2))
        smp = ctx.enter_context(tc.tile_pool(name="smp", bufs=1))
        psp = ctx.enter_context(tc.tile_pool(name="psp", bufs=3, space="PSUM"))
        dram = ctx.enter_context(tc.tile_pool(name="dram", bufs=2, space="DRAM"))

        # --- persistent tiles -------------------------------------------
        uhb = [persist.tile([P, ROW], BF16, name=f"uhb{k}", tag=f"uhb{k}")
               for k in range(NBLK)]
        w_fo = None
        if R > 1:
            w_fo = persist.tile([P, ROW], BF16, name="w_fo")
        c0 = persist.tile([P, 1], BF16, name="c0")
        nc.vector.memset(c0, 1.0 / OUT_NODES)
        # one-hot stationaries for squash in the chunk-major p-mapping:
        # p = q*32 + f*2 + hi  ->  m = q*2 + hi = (p//32)*2 + p%2
        pidx = np.arange(P)
        m_of_p = (pidx // 32) * 2 + (pidx % 2)
        oh1_d = nc.inline_tensor(
            (m_of_p[:, None] == np.arange(8)[None, :]).astype('bfloat16'),
            name="oh1d")
        oh2_d = nc.inline_tensor(
            (np.arange(8)[:, None] == m_of_p[None, :]).astype('bfloat16'),
            name="oh2d")
        oh1 = persist.tile([P, 8], BF16, name="oh1")
        nc.sync.dma_start(oh1, oh1_d.ap())
        oh2 = persist.tile([8, P], BF16, name="oh2")
        nc.sync.dma_start(oh2, oh2_d.ap())

        w_acc_prev = None

        def fhalf(tile_, h):
            # strided f-half view: [p, 4 chunks, 2048] (f<8 or f>=8)
            return tile_.rearrange("p (q c) -> p q c", c=QT)[
                :, :, h * (QT // 2):(h + 1) * (QT // 2)]

        for t in range(1, R + 1):
            ar_in = dram.tile([ROW], BF16, tag="ar_in")
            ar_v = ar_in.rearrange("(q f c) -> q f c", q=NQT, f=F_SIZE)
            if t == 1:
                # ---- pass 1: stream, convert, and matmul per chunk-set ----
                for q in range(NQT):
                    for blk in range(NBLK):
                        st = scp.tile([P, QT], F32, tag="sc", name="st")
                        nc.sync.dma_start(
                            st, uh_t[blk, :, q * QT:(q + 1) * QT])
                        dst = uhb[blk][:, q * QT:(q + 1) * QT].rearrange(
                            "p (f c) -> p f c", f=F_SIZE)
                        nc.vector.tensor_copy(
                            dst, st.rearrange("p (o f) -> p f o", f=F_SIZE))
                    # per f-quad: one [1,1024] psum, 4 segment-groups
                    for fq in range(F_SIZE // 4):
                        ps = psp.tile([1, O], F32, tag="ps1", name="ps")
                        for j in range(4):
                            f = fq * 4 + j
                            for blk in range(NBLK):
                                nc.tensor.matmul(
                                    ps[:, j * SEG:(j + 1) * SEG], c0,
                                    uhb[blk][:, q * QT + f * SEG:
                                             q * QT + (f + 1) * SEG],
                                    start=(blk == 0), stop=(blk == NBLK - 1),
                                    skip_group_check=True)
                        fl = smp.tile([1, O], BF16, tag="bfl", bufs=2,
                                      name="fl")
                        nc.scalar.copy(fl, ps)
                        # quad spans contiguous chunk-major cols
                        nc.sync.dma_start(
                            ar_in[q * QT + fq * O:q * QT + (fq + 1) * O], fl)
                rbs = [c0] * NBLK
                e2s = None
            else:
                # ---- passes >= 2: b, e, rinv per block from SBUF cache ----
                rbs, e2s = [], []
                for blk in range(NBLK):
                    tmA = scp.tile([P, H], BF16, tag="sc", name="tmA")
                    tmB = scp.tile([P, H], BF16, tag="sc", name="tmB")
                    tmAv = tmA.rearrange("p (q c) -> p q c", c=QT // 2)
                    tmBv = tmB.rearrange("p (q c) -> p q c", c=QT // 2)
                    nc.vector.tensor_mul(tmAv, fhalf(uhb[blk], 0),
                                         fhalf(w_fo, 0))
                    nc.vector.tensor_mul(tmBv, fhalf(uhb[blk], 1),
                                         fhalf(w_fo, 1))
                    # halving tree over f (in-place, all on DVE)
                    nc.vector.tensor_add(tmA, tmA, tmB)
                    nc.vector.tensor_add(
                        tmAv[:, :, 0:1024], tmAv[:, :, 0:1024],
                        tmAv[:, :, 1024:2048])
                    nc.vector.tensor_add(
                        tmAv[:, :, 0:512], tmAv[:, :, 0:512],
                        tmAv[:, :, 512:1024])
                    b = smp.tile([P, O], BF16, tag="bfl", bufs=2,
                                 name="b")
                    nc.vector.tensor_add(
                        b.rearrange("p (q c) -> p q c", c=SEG),
                        tmAv[:, :, 0:SEG], tmAv[:, :, SEG:2 * SEG])
                    e2 = smp.tile([P, O], BF16, tag=f"e2_{blk}", name="e2")
                    den = smp.tile([P, 1], F32, tag="den", name="den")
                    nc.scalar.activation(e2, b, AF.Exp, accum_out=den)
                    rinv = smp.tile([P, 1], F32, tag="rinv", name="rinv")
                    nc.vector.reciprocal(rinv, den)
                    rb = smp.tile([P, 1], BF16, tag=f"rb_{blk}", name="rb")
                    nc.vector.tensor_copy(rb, rinv)
                    rbs.append(rb)
                    e2s.append(e2)

                # ---- s partials: per f-plane, PSUM-accumulate over blocks
                for fp in range(F_SIZE // 2):
                    pss = [psp.tile([1, O], F32, tag="ps1", name="ps",
                                    padded_shape=[P, O]) for _ in range(2)]
                    for blk in range(NBLK):
                        # pt2 flat [p, (j, o)]: both planes of the pair,
                        # o-contiguous per plane for the 512-wide windows
                        pt2 = scp.tile([P, 2 * O], BF16, tag="sc", name="pt2")
                        nc.vector.tensor_mul(
                            pt2.rearrange("p (j q c) -> p q j c",
                                          j=2, c=SEG),
                            uhb[blk].rearrange("p (q c) -> p q c", c=QT)[
                                :, :, 2 * fp * SEG:(2 * fp + 2) * SEG]
                            .rearrange("p q (j c) -> p q j c", c=SEG),
                            e2s[blk].rearrange("p (q c) -> p q c", c=SEG)[
                                :, :, None, :].broadcast_to([P, NQT, 2, SEG]))
                        for j in range(2):
                            for w0 in (0, 512):
                                nc.tensor.matmul(
                                    pss[j][:, w0:w0 + 512], rbs[blk],
                                    pt2[:, j * O + w0:j * O + w0 + 512],
                                    start=(blk == 0), stop=(blk == NBLK - 1),
                                    skip_group_check=True)
                    for j in range(2):
                        fl = smp.tile([1, O], BF16, tag="bfl", bufs=2,
                                      name="fl")
                        nc.scalar.copy(fl, pss[j])
                        # s[f, o] o-ordered -> chunk-major ar positions
                        nc.sync.dma_start(ar_v[:, 2 * fp + j], fl)

            ar_out = dram.tile([ROW], BF16, tag="ar_out")
            nc.gpsimd.collective_compute(
                "AllReduce", AO.add, replica_groups=rg,
                ins=[ar_in.opt()], outs=[ar_out.opt()],
            )

            # ---- squash in p-major layout: p = q*32+f*2+hi, free = lo ----
            sld = smp.tile([P, P], BF16, tag="sld", name="sld")
            nc.sync.dma_start(sld, ar_out.rearrange("(p q) -> p q", p=P))
            ssq = smp.tile([P, P], BF16, tag="ssq", name="ssq")
            nc.vector.tensor_mul(ssq, sld, sld)
            sqps = psp.tile([8, P], F32, tag="sqps", bufs=1, name="sqps")
            nc.tensor.matmul(sqps, oh1, ssq, start=True, stop=True,
                             skip_group_check=True)
            sq = smp.tile([8, P], BF16, tag="sq", name="sq")
            nc.scalar.copy(sq, sqps)
            # sqrt(sq) via exp(0.5*ln) + one Newton step (exp/ln table set)
            lnq = smp.tile([8, P], BF16, tag="lnq", name="lnq")
            nc.scalar.activation(lnq, sq, AF.Ln)
            y = smp.tile([8, P], BF16, tag="y", name="y")
            nc.scalar.activation(y, lnq, AF.Exp, scale=0.5)
            d1 = smp.tile([8, P], BF16, tag="t1", name="d1")
            nc.vector.tensor_scalar(d1, sq, 1.0, None, AO.add)
            rd = smp.tile([8, P], BF16, tag="ry", name="rd")
            with nc.allow_low_precision(reason="bf16 squash chain"):
                nc.vector.reciprocal(rd, d1)       # rd = 1/(1+sq)
            sc = smp.tile([8, P], BF16, tag="sq", name="sc")
            nc.vector.tensor_mul(sc, y, rd)        # sqrt(sq)/(1+sq)
            srps = psp.tile([P, P], F32, tag="srps", bufs=1, name="srps")
            nc.tensor.matmul(srps, oh2, sc, start=True, stop=True,
                             skip_group_check=True)
            v_sb = smp.tile([P, P], BF16, tag="v_sb", name="v_sb")
            nc.vector.tensor_mul(v_sb, sld, srps)

            if t == R:
                nc.sync.dma_start(v_out, v_sb)
            else:
                w_acc = smp.tile([P, P], BF16, tag="w_acc", bufs=2,
                                 name="w_acc")
                if t == 1:
                    nc.scalar.copy(w_acc, v_sb)
                else:
                    nc.vector.tensor_add(w_acc, w_acc_prev, v_sb)
                w_acc_prev = w_acc
                w_dram = dram.tile([ROW], BF16, tag="w_dram")
                nc.sync.dma_start(
                    w_dram.rearrange("(p q) -> p q", p=P), w_acc)
                wd_b = w_dram.unsqueeze(0)
                wd_v = wd_b.rearrange("x (q c) -> x q c", c=QT)
                for h in (0, 1):
                    sl = slice(h * (QT // 2), (h + 1) * (QT // 2))
                    nc.sync.dma_start(
                        fhalf(w_fo, h),
                        wd_v[:, :, sl].broadcast_to([P, NQT, QT // 2]))


def _build(routing_num: int):
    R = int(routing_num)
    assert R >= 1
    nc = bacc.Bacc(
        "TRN2", target_bir_lowering=False, debug=False, num_devices=CORES)
    uh = nc.dram_tensor("uh", [I_LOC, ROW], F32, kind="ExternalInput")
    v_out = nc.dram_tensor("v_out", [P, P], BF16, kind="ExternalOutput")
    rg = [list(range(CORES))]
    with tile.TileContext(nc) as tc:
        _body(nc, tc, uh.ap(), v_out.ap(), R, rg)
    nc.compile()
    return nc


_CACHE: dict = {}


def _get_nc(routing_num: int):
    R = int(routing_num)
    if R not in _CACHE:
        _CACHE[R] = _build(R)
    return _CACHE[R]


def _shard(u_hat: np.ndarray):
    uh = np.ascontiguousarray(np.asarray(u_hat, dtype=np.float32))
    assert uh.shape == (IN_NODES * OUT_NODES, F_SIZE), uh.shape
    uh = uh.reshape(IN_NODES, ROW)
    return [
        {"uh": np.ascontiguousarray(uh[k * I_LOC:(k + 1) * I_LOC])}
        for k in range(CORES)
    ]


def run(u_hat, routing_num, trace=False):
    nc = _get_nc(routing_num)
    in_maps = _shard(u_hat)
    res = bass_utils.run_bass_kernel_spmd(
        nc, in_maps, core_ids=list(range(CORES)), trace=trace)
    return res


def _unpack(v_pm) -> np.ndarray:
    # [128,128] p-major bf16, p = q*32 + f*2 + hi, free = lo
    # o = q*256 + hi*128 + lo  ->  [1024, 16] f32
    v = np.asarray(v_pm).astype(np.float32).reshape(NQT, F_SIZE, 2, P)
    return np.ascontiguousarray(
        v.transpose(0, 2, 3, 1).reshape(OUT_NODES, F_SIZE))


def kernel(u_hat, routing_num):
    res = run(u_hat, routing_num, trace=False)
    return _unpack(res.results[0]["v_out"])
